# revision 11
# baseline (speedup 1.0000x reference)
"""Multi-head attention (B=4, S=2048, E=1024, H=16, D=64) on 8 Trainium2 cores.

Sharding: core c handles batch b=c//2 and head-group g=c%2 (8 of the 16 heads).
Each core computes, for its (batch, head-group):
  qk_T = (x_b @ w_qk + b_qk)^T        transposed QK projection  [1024, S]
  V    = x_b @ w_v + b_v              natural V projection      [S, 512]
  per head: S_T = K @ Q^T (f32r matmuls), P^T = exp(S_T/8),
            attn' = [V|1]^T @ P^T  (ones column gives softmax denominators)
            attn_T = attn'[0:64] * (1/attn'[64]) broadcast
  out_T partial = (attn_T^T @ w_out_rows)^T                     [1024, S]
Host sums the two head-group partials per batch, transposes, adds b_out.

All matmuls run in float32r (TF32-like: ~1.5e-4 rel err, full PE speed).
"""
import os
import sys

sys.path.insert(0, "/opt/trn_rl_repo")

import numpy as np

import concourse.bass as bass
import concourse.mybir as mybir
import concourse.tile as tile
from concourse import bacc
from concourse.bass_utils import run_bass_kernel_spmd

B, S, E, H, D = 4, 2048, 1024, 16, 64
HPC = 8            # heads per core
NCORES = 8
P = 128
f32 = mybir.dt.float32
f32r = mybir.dt.float32r
AF = mybir.ActivationFunctionType
SCALE = 1.0 / 8.0  # 1/sqrt(D)

# module-level stash so test.py can reuse the compiled kernel / results
_BUILD_CACHE = {}
LAST_RESULTS = None


def build_nc(s=S, repeat=1):
    """Build + compile the per-core Bass program. Same NEFF for all 8 cores."""
    nsq = s // 512        # 512-wide s chunks
    nst = s // P          # 128-wide s tiles
    nc = bacc.Bacc("TRN2", target_bir_lowering=False, debug=False,
                   num_devices=NCORES)

    xT = nc.dram_tensor("xT", [E, s], f32r, kind="ExternalInput").ap()
    w_qk = nc.dram_tensor("w_qk", [E, HPC * 128], f32r, kind="ExternalInput").ap()
    b_qk = nc.dram_tensor("b_qk", [HPC * 128, 1], f32, kind="ExternalInput").ap()
    w_v = nc.dram_tensor("w_v", [E, HPC * D], f32r, kind="ExternalInput").ap()
    b_v = nc.dram_tensor("b_v", [P, HPC * D], f32, kind="ExternalInput").ap()
    w_out = nc.dram_tensor("w_out", [HPC * D, E], f32r, kind="ExternalInput").ap()
    outT = nc.dram_tensor("outT", [E, s], f32, kind="ExternalOutput").ap()
    scratch = nc.dram_tensor("scratch", [HPC, s], f32).ap()  # denominators bounce

    xT_r = xT.rearrange("(ko p) s -> p ko s", p=P)        # [128, 8, s]
    wqk_r = w_qk.rearrange("(ko p) f -> p ko f", p=P)     # [128, 8, 1024]
    wv_r = w_v.rearrange("(ko p) f -> p ko f", p=P)       # [128, 8, 512]
    bqk_r = b_qk.rearrange("(m p) one -> p (m one)", p=P)  # [128, 8]
    wo_r = w_out.rearrange("(j p) f -> p j f", p=P)       # [128, 4, 1024]
    outT_r = outT.rearrange("(m p) s -> p m s", p=P)      # [128, 8, s]

    with tile.TileContext(nc) as tc:
        def body():
            from contextlib import ExitStack
            with ExitStack() as outer:
                persist = outer.enter_context(tc.tile_pool(name="persist", bufs=1))
                qT2 = persist.tile([P, HPC // 2, s], f32r)   # [64*2 packed, pair, s]
                kT2 = persist.tile([P, HPC // 2, s], f32r)
                v_sb = persist.tile([P, nst, HPC, D + 1], f32r)  # V' with ones col
                bqk_sb = persist.tile([P, HPC], f32)
                bv_sb = persist.tile([P, HPC, D], f32)
                nc.sync.dma_start(bqk_sb[:], bqk_r)
                nc.sync.dma_start(bv_sb[:], b_v.rearrange("p (h d) -> p h d", d=D))
                for st in range(nst):
                    nc.vector.memset(v_sb[:, st, :, D:D + 1].bitcast(f32), 1.0)

                # ---- Phase A/B: projections, streaming x^T in 512-col chunks
                with ExitStack() as ab:
                    xpool = ab.enter_context(tc.tile_pool(name="x", bufs=2))
                    wqk_pool = ab.enter_context(tc.tile_pool(name="wqk", bufs=1))
                    wv_pool = ab.enter_context(tc.tile_pool(name="wv", bufs=1))
                    psA = ab.enter_context(
                        tc.tile_pool(name="psA", bufs=3, space="PSUM"))
                    psB = ab.enter_context(
                        tc.tile_pool(name="psB", bufs=2, space="PSUM"))
                    wqk_sb = wqk_pool.tile([P, 8, HPC * 128], f32r)
                    wv_sb = wv_pool.tile([P, 8, HPC * D], f32r)
                    nc.sync.dma_start(wqk_sb[:], wqk_r)
                    nc.sync.dma_start(wv_sb[:], wv_r)

                    for q in range(nsq):
                        sq = slice(q * 512, (q + 1) * 512)
                        xt = xpool.tile([P, 8, 512], f32r)
                        nc.sync.dma_start(xt[:], xT_r[:, :, sq])
                        for m in range(HPC):  # qk feature tiles of 128
                            ps = psA.tile([P, 512], f32)
                            for k in range(8):
                                nc.tensor.matmul(
                                    ps[:], lhsT=wqk_sb[:, k, m * P:(m + 1) * P],
                                    rhs=xt[:, k, :],
                                    start=(k == 0), stop=(k == 7))
                            dst = qT2 if m % 2 == 0 else kT2
                            nc.vector.tensor_scalar_add(
                                dst[:, m // 2, sq], ps[:], bqk_sb[:, m:m + 1])
                        for stl in range(4):  # s tiles of 128 in this chunk
                            st = q * 4 + stl
                            ps = psB.tile([P, 512], f32)
                            for k in range(8):
                                nc.tensor.matmul(
                                    ps[:], lhsT=xt[:, k, stl * P:(stl + 1) * P],
                                    rhs=wv_sb[:, k, :],
                                    start=(k == 0), stop=(k == 7))
                            nc.vector.tensor_add(
                                v_sb[:, st, :, 0:D],
                                ps.rearrange("p (h d) -> p h d", d=D),
                                bv_sb[:])

                # ---- Phases C+D share the attnT pool (opened after A/B frees
                # x/w space)
                cd = outer.enter_context(ExitStack())
                attnT_pool = cd.enter_context(tc.tile_pool(name="attnT", bufs=1))
                attnT = attnT_pool.tile([P, HPC * D // P, s], f32r)

                # ---- Phase C: attention per head
                with ExitStack() as c:
                    psS = c.enter_context(
                        tc.tile_pool(name="psS", bufs=2, space="PSUM"))
                    psAt = c.enter_context(
                        tc.tile_pool(name="psAt", bufs=4, space="PSUM"))
                    ppool = c.enter_context(tc.tile_pool(name="pT", bufs=3))
                    npool = c.enter_context(tc.tile_pool(name="norm", bufs=2))
                    spool = c.enter_context(tc.tile_pool(name="asb", bufs=2))
                    for i in range(HPC):
                        u, poff = i // 2, (i % 2) * 64
                        QT = qT2[poff:poff + 64, u, :]
                        KT = kT2[poff:poff + 64, u, :]
                        at_tiles = [psAt.tile([D + 1, 512], f32,
                                              name=f"at{q}", tag=f"at{q}",
                                              bufs=1)
                                    for q in range(nsq)]
                        gw = min(2, nsq)  # sq chunks per exp group
                        for skt in range(nst):
                            for h2 in range(nsq // gw):
                                ps_s = psS.tile([P, gw * 512], f32,
                                                name="ps_s", tag="ps_s")
                                for j in range(gw):
                                    q = h2 * gw + j
                                    nc.tensor.matmul(
                                        ps_s[:, j * 512:(j + 1) * 512],
                                        lhsT=KT[:, skt * P:(skt + 1) * P],
                                        rhs=QT[:, q * 512:(q + 1) * 512],
                                        start=True, stop=True)
                                pT = ppool.tile([P, gw * 512], f32r,
                                                name="pT", tag="pT")
                                nc.scalar.activation(pT[:], ps_s[:], AF.Exp,
                                                     scale=SCALE)
                                for j in range(gw):
                                    q = h2 * gw + j
                                    nc.tensor.matmul(
                                        at_tiles[q][:],
                                        lhsT=v_sb[:, skt, i, :],
                                        rhs=pT[:, j * 512:(j + 1) * 512],
                                        start=(skt == 0),
                                        stop=(skt == nst - 1))
                        # evacuate attn accumulators to SBUF (frees PSUM banks
                        # so the next head's PV can proceed during this norm)
                        attn_sb = spool.tile([D + 1, s], f32)
                        for q in range(nsq):
                            nc.vector.tensor_copy(
                                attn_sb[:, q * 512:(q + 1) * 512],
                                at_tiles[q][:])
                        # softmax normalization: recip of denominators (row 64),
                        # broadcast over 64 partitions via DRAM bounce
                        recip = npool.tile([1, s], f32, tag="recip")
                        nc.vector.reciprocal(recip[:], attn_sb[D:D + 1, :])
                        nc.sync.dma_start(scratch[i:i + 1, :], recip[:])
                        bc = npool.tile([64, s], f32, tag="bc")
                        nc.sync.dma_start(
                            bc[:], scratch[i:i + 1, :].partition_broadcast(64)
                            .rearrange("p one s -> p (one s)"))
                        nc.vector.tensor_mul(
                            attnT[poff:poff + 64, i // 2, :],
                            attn_sb[0:D, :], bc[:])

                # ---- Phase D: output projection (partial; host sums pairs)
                with ExitStack() as d:
                    wo_pool = d.enter_context(tc.tile_pool(name="wo", bufs=1))
                    psD = d.enter_context(
                        tc.tile_pool(name="psD", bufs=2, space="PSUM"))
                    opool = d.enter_context(tc.tile_pool(name="osb", bufs=2))
                    wo_sb = wo_pool.tile([P, 4, E], f32r)
                    nc.sync.dma_start(wo_sb[:], wo_r)
                    for m in range(8):
                        ps_o = psD.tile([P, s], f32)
                        for k in range(4):
                            for n4 in range(nsq):
                                nc.tensor.matmul(
                                    ps_o[:, n4 * 512:(n4 + 1) * 512],
                                    lhsT=wo_sb[:, k, m * P:(m + 1) * P],
                                    rhs=attnT[:, k, n4 * 512:(n4 + 1) * 512],
                                    start=(k == 0), stop=(k == 3))
                        o_sb = opool.tile([P, s], f32)
                        nc.vector.tensor_copy(o_sb[:], ps_o[:])
                        nc.sync.dma_start(outT_r[:, m, :], o_sb[:])

        if repeat > 1:
            with tc.For_i(0, repeat, 1):
                body()
        else:
            body()

    nc.compile()
    return nc


def _get_nc(s=S, repeat=1):
    key = (s, repeat)
    if key not in _BUILD_CACHE:
        _BUILD_CACHE[key] = build_nc(s=s, repeat=repeat)
    return _BUILD_CACHE[key]


def shard_inputs(x, w_qkv, b_qkv, w_out, b_out):
    """Host-side sharding: per-core input maps."""
    in_maps = []
    for c in range(NCORES):
        b, g = c // 2, c % 2
        heads = [g * HPC + i for i in range(HPC)]
        # qk columns, pair-interleaved: block 2u = q cols of head pair u,
        # block 2u+1 = k cols of head pair u
        qk_cols, qk_bias = [], []
        for u in range(HPC // 2):
            hA, hB = heads[2 * u], heads[2 * u + 1]
            for off in (0, 64):  # 0: q, 64: k
                for h in (hA, hB):
                    qk_cols.append(w_qkv[:, h * 192 + off:h * 192 + off + 64])
                    qk_bias.append(b_qkv[h * 192 + off:h * 192 + off + 64])
        w_qk_c = np.ascontiguousarray(np.concatenate(qk_cols, axis=1))
        b_qk_c = np.ascontiguousarray(
            np.concatenate(qk_bias)[:, None].astype(np.float32))
        w_v_c = np.ascontiguousarray(np.concatenate(
            [w_qkv[:, h * 192 + 128:h * 192 + 192] for h in heads], axis=1))
        b_v_c = np.ascontiguousarray(np.broadcast_to(np.concatenate(
            [b_qkv[h * 192 + 128:h * 192 + 192] for h in heads])[None, :],
            (P, HPC * D)).astype(np.float32))
        w_out_c = np.ascontiguousarray(np.concatenate(
            [w_out[h * D:(h + 1) * D, :] for h in heads], axis=0))
        xT_c = np.ascontiguousarray(x[b].T)
        in_maps.append({
            "xT": xT_c, "w_qk": w_qk_c, "b_qk": b_qk_c,
            "w_v": w_v_c, "b_v": b_v_c, "w_out": w_out_c,
        })
    return in_maps


def unshard_output(results, b_out):
    out = np.empty((B, S, E), dtype=np.float32)
    for b in range(B):
        acc = results[2 * b]["outT"] + results[2 * b + 1]["outT"]
        out[b] = acc.T + b_out
    return out


def kernel(x, w_qkv, b_qkv, w_out, b_out):
    global LAST_RESULTS
    x = np.asarray(x, dtype=np.float32)
    w_qkv = np.asarray(w_qkv, dtype=np.float32)
    b_qkv = np.asarray(b_qkv, dtype=np.float32)
    w_out = np.asarray(w_out, dtype=np.float32)
    b_out = np.asarray(b_out, dtype=np.float32)

    nc = _get_nc()
    in_maps = shard_inputs(x, w_qkv, b_qkv, w_out, b_out)
    res = run_bass_kernel_spmd(nc, in_maps, list(range(NCORES)))
    LAST_RESULTS = res
    return unshard_output(res.results, b_out)
